# revision 30
# baseline (speedup 1.0000x reference)
"""Trainium2 Bass kernel for CausalSelfAttention (RoPE + ALiBi + causal mask).

Sharding: 16 heads tensor-parallel across 8 NeuronCores (2 heads/core).
Per core:
  phase 1: qkv projection from replicated x^T in bf16 (halves the DMA
           stream; PSUM accumulation stays f32). RoPE applied on the fly
           via cross-partition DVE multiplies against a sign-folded sin
           table (no PE rotation matmul). q^T,k^T kept in SBUF [d, t].
  phase 2: attention per (batch, head) in transposed layout
           S^T[j, i] = k^T.T @ q^T; ALiBi+mask added by DVE into SBUF
           (frees the PSUM score bank after one op); exp on ScalarE;
           row-sums via ones-matmul into a shared per-(b,icx) PSUM bank;
           y^T accumulated on TensorE; reciprocal broadcast on GpSimd.
           Diagonal-block offsets are clamped to keep matmul free dims
           >= 256 (f32r runs 4x slower below that); over-computed columns
           hit an all-NEG bias slot and exp to 0.
  phase 3: out partial = y @ W_proj (rows of the core's heads),
           interleaved with phase 2 per query chunk.
Host: sums the 8 partial outputs.

Attention matmuls run in float32r (TF32-like, full PE rate at free dim
>= 256); the qkv projection runs in bf16. DMA issue order is consumption
order so the first chunk's weights/x lead and phase-2/3 constants trail.
"""

import math
from contextlib import ExitStack

import numpy as np

import concourse.bass as bass
import concourse.mybir as mybir
import concourse.tile as tile
from concourse import bacc
from concourse.bass_utils import run_bass_kernel_spmd

B, T, DM = 2, 2048, 2048
H, HD = 16, 128
ROWS = B * T                      # 4096
NCORES = 8
HPC = H // NCORES                 # 2 heads per core
ROPE_THETA = 10000.0
SQHD = math.sqrt(HD)
M_OFF = 18.0                      # softmax stability offset
NEG = -1.0e30

TCH = 512                         # t-chunk width in phase 1
NCH = ROWS // TCH                 # 8
CT = DM // 128                    # 16 contraction tiles
NT = T // 128                     # 16 key/query tiles per batch
IC = 512                          # query chunk in phase 2
NIC = T // IC                     # 4

F32 = mybir.dt.float32
F32R = mybir.dt.float32r
BF16 = mybir.dt.bfloat16
MULT = mybir.AluOpType.mult
ADD = mybir.AluOpType.add
EXP = mybir.ActivationFunctionType.Exp


def build_program(phases="123", loop_n=1):
    nc = bacc.Bacc("TRN2", target_bir_lowering=False, debug=False,
                   num_devices=NCORES)
    xT = nc.dram_tensor("xT", [DM, ROWS], BF16, kind="ExternalInput").ap()
    wq = nc.dram_tensor("wq", [DM, HPC * HD], BF16, kind="ExternalInput").ap()
    wk = nc.dram_tensor("wk", [DM, HPC * HD], BF16, kind="ExternalInput").ap()
    wv = nc.dram_tensor("wv", [DM, HPC * HD], BF16, kind="ExternalInput").ap()
    wp = nc.dram_tensor("wp", [HPC * HD, DM], F32, kind="ExternalInput").ap()
    cosT = nc.dram_tensor("cosT", [128, T], F32, kind="ExternalInput").ap()
    sinT = nc.dram_tensor("sinT", [128, T], F32, kind="ExternalInput").ap()
    biasd = nc.dram_tensor("biasd", [128, HPC, 17, 128], F32,
                           kind="ExternalInput").ap()
    ones128 = nc.dram_tensor("ones128", [128, 1], F32, kind="ExternalInput").ap()
    out = nc.dram_tensor("out", [ROWS, DM], F32, kind="ExternalOutput").ap()

    xT3 = xT.rearrange("(o p) t -> p o t", p=128)

    with tile.TileContext(nc) as tc, ExitStack() as ctx:
        const = ctx.enter_context(tc.tile_pool(name="const", bufs=1))
        qkp = ctx.enter_context(tc.tile_pool(name="qk", bufs=1))

        q_sb = [qkp.tile([128, ROWS], F32R, tag=f"q{e}", name=f"q{e}")
                for e in range(HPC)]
        k_sb = [qkp.tile([128, ROWS], F32R, tag=f"k{e}", name=f"k{e}")
                for e in range(HPC)]
        v_keep = qkp.tile([128, B * NT, HPC * HD], F32R, tag="vk", name="vk")

        cos_sb = const.tile([128, T], F32, tag="cos")
        sin_sb = const.tile([128, T], F32, tag="sin")
        ones128_sb = const.tile([128, 1], F32R, tag="o128")
        bias_sb = const.tile([128, HPC, 17, 128], F32, tag="bias")
        wp_sb = const.tile([128, HPC, DM], F32R, tag="wp")

        if loop_n > 1:
            # timing mode: run the whole body loop_n times on-device
            ctx.enter_context(tc.For_i(0, loop_n, 1))

        # ---------------- phase 1: qkv + rope ----------------
        with tc.tile_pool(name="w1", bufs=1) as wpool, \
             tc.tile_pool(name="xt", bufs=24) as xpool, \
             tc.tile_pool(name="rope", bufs=5) as rpool, \
             tc.tile_pool(name="p1", bufs=6, space="PSUM") as ps1, \
             tc.tile_pool(name="pv", bufs=2, space="PSUM") as psv:
            wq_sb = wpool.tile([128, CT, HPC * HD], BF16, tag="wq")
            wk_sb = wpool.tile([128, CT, HPC * HD], BF16, tag="wk")
            wv_sb = wpool.tile([128, CT, HPC * HD], BF16, tag="wv")
            # DMA issue order is consumption order: everything chunk 0 needs
            # (all weight groups + its 16 x tiles, interleaved) leads; then
            # rope tables; phase-2/3 constants trail behind
            xts0 = []
            for g in range(4):
                cg = slice(g * 4, (g + 1) * 4)
                nc.sync.dma_start(wq_sb[:, cg, :], wq.rearrange("(o p) e -> p o e", p=128)[:, cg, :])
                nc.sync.dma_start(wk_sb[:, cg, :], wk.rearrange("(o p) e -> p o e", p=128)[:, cg, :])
                for ci in range(g * 4, g * 4 + 4):
                    xt0 = xpool.tile([128, TCH], BF16, tag="xt")
                    nc.sync.dma_start(xt0[:], xT3[:, ci, 0:TCH])
                    xts0.append(xt0)
            nc.sync.dma_start(cos_sb[:], cosT)
            nc.sync.dma_start(sin_sb[:], sinT)
            nc.sync.dma_start(ones128_sb[:], ones128.bitcast(F32R))
            for g in range(4):
                cg = slice(g * 4, (g + 1) * 4)
                nc.sync.dma_start(wv_sb[:, cg, :], wv.rearrange("(o p) e -> p o e", p=128)[:, cg, :])
            nc.sync.dma_start(bias_sb[:], biasd)
            nc.sync.dma_start(wp_sb[:], wp.rearrange("(o p) e -> p o e", p=128).bitcast(F32R))

            for tchunk in range(NCH):
                t0 = tchunk * TCH
                if tchunk == 0:
                    xts = xts0
                else:
                    xts = []
                    for ci in range(CT):
                        xt_t = xpool.tile([128, TCH], BF16, tag="xt")
                        nc.sync.dma_start(xt_t[:], xT3[:, ci, t0:t0 + TCH])
                        xts.append(xt_t)

                cs = slice(t0 % T, t0 % T + TCH)
                for dst, w_sb in ((q_sb, wq_sb), (k_sb, wk_sb)):
                    for et in range(HPC):
                        ps_q = ps1.tile([128, TCH], F32, tag="psq")
                        for ci in range(CT):
                            nc.tensor.matmul(ps_q[:],
                                             w_sb[:, ci, et * HD:(et + 1) * HD],
                                             xts[ci][:],
                                             start=(ci == 0), stop=(ci == CT - 1))
                        qraw = rpool.tile([128, TCH], F32R, tag="qraw")
                        nc.scalar.copy(qraw[:], ps_q[:])
                        # rotate-half via cross-partition DVE ops; sin_sb rows
                        # 0:64 hold -sin so tmp = rot(q) * sin in two halves
                        tmp = rpool.tile([128, TCH], F32, tag="tmp")
                        # both INPUTS share a base partition (verifier
                        # requirement); only the output is offset. sin rows
                        # 64:128 hold -sin_h, rows 0:64 hold +sin_h.
                        nc.vector.tensor_mul(tmp[0:64, :], qraw[64:128, :],
                                             sin_sb[64:128, cs])
                        nc.vector.tensor_mul(tmp[64:128, :], qraw[0:64, :],
                                             sin_sb[0:64, cs])
                        dcols = dst[et][:, t0:t0 + TCH]
                        nc.vector.tensor_mul(dcols, qraw[:], cos_sb[:, cs])
                        nc.vector.tensor_add(dcols, dcols, tmp[:])

                for tt in range(TCH // 128):
                    ps_vt = psv.tile([128, HPC * HD], F32, tag="psv")
                    for ci in range(CT):
                        nc.tensor.matmul(ps_vt[:],
                                         xts[ci][:, tt * 128:(tt + 1) * 128],
                                         wv_sb[:, ci, :],
                                         start=(ci == 0), stop=(ci == CT - 1))
                    nc.scalar.copy(v_keep[:, t0 // 128 + tt, :], ps_vt[:])

        # ---------------- phase 2+3: attention + projection ----------------
        do2 = "2" in phases
        with tc.tile_pool(name="wt", bufs=12) as wpool2, \
             tc.tile_pool(name="yb", bufs=2) as ypool, \
             tc.tile_pool(name="sm", bufs=2) as smpool, \
             tc.tile_pool(name="ost", bufs=4) as ostp, \
             tc.tile_pool(name="pssc", bufs=2, space="PSUM") as pssc, \
             tc.tile_pool(name="psacc", bufs=2, space="PSUM") as psacc, \
             tc.tile_pool(name="psm", bufs=2, space="PSUM") as psmisc, \
             tc.tile_pool(name="pso", bufs=2, space="PSUM") as pso:
            for b in range(B if do2 else 0):
                y_b = ypool.tile([128, HPC, T], F32R, tag="yb")
                for icx in range(NIC):
                    i0 = b * T + icx * IC
                    jt_hi = (icx + 1) * (IC // 128)
                    for hi in range(HPC):
                        ps_sum = psmisc.tile([1, IC], F32, tag="psm")
                        ps_y = psacc.tile([128, IC], F32, tag="psy")
                        for jt in range(jt_hi):
                            # skip fully-masked query columns (i-tile >= jt),
                            # clamped to keep free dim >= 256 (f32r full rate);
                            # the over-computed columns hit the all-NEG bias
                            # slot and exp to exactly 0.
                            o = min(max(0, jt * 128 - icx * IC), IC - 256)
                            n = IC - o
                            ps_sc = pssc.tile([128, IC], F32, tag="pssc")
                            nc.tensor.matmul(
                                ps_sc[:, o:],
                                k_sb[hi][:, b * T + jt * 128: b * T + (jt + 1) * 128],
                                q_sb[hi][:, i0 + o:i0 + IC],
                                start=True, stop=True)
                            # bias slot s = (i-tile - jt) + 1; slot 0 = all-NEG
                            d0 = (icx * IC + o) // 128 - jt + 1
                            ps3 = ps_sc[:, o:].rearrange("p (a c) -> p a c", c=128)
                            w_t = wpool2.tile([128, IC], F32R, tag="wt")
                            w3 = w_t[:, o:].rearrange("p (a c) -> p a c", c=128)
                            # biased scores land in SBUF so the PSUM score
                            # bank frees after one DVE op, not after exp
                            nc.vector.scalar_tensor_tensor(
                                out=w3, in0=ps3, scalar=1.0,
                                in1=bias_sb[:, hi, d0:d0 + n // 128, :],
                                op0=MULT, op1=ADD)
                            nc.scalar.activation(w_t[:, o:], w_t[:, o:], EXP,
                                                 bias=0.0, scale=1.0 / SQHD)
                            nc.tensor.matmul(ps_y[:, o:], v_keep[:, b * NT + jt, hi * HD:(hi + 1) * HD], w_t[:, o:],
                                             start=(jt == 0), stop=(jt == jt_hi - 1))
                            nc.tensor.matmul(ps_sum[0:1, o:], ones128_sb[:], w_t[:, o:],
                                             start=(jt == 0), stop=(jt == jt_hi - 1))
                        recip = smpool.tile([1, IC], F32R, tag="recip")
                        with nc.allow_low_precision(reason="f32r is 4-byte"):
                            nc.vector.reciprocal(recip[:], ps_sum[0:1, :])
                        # broadcast recip down 128 partitions on the (idle)
                        # gpsimd engine; frees PE of the ones1 matmul and ACT
                        # of the staging copy
                        bca = smpool.tile([128, IC], F32R, tag="bca")
                        nc.gpsimd.partition_broadcast(bca[:], recip[0:1, :])
                        nc.vector.tensor_mul(y_b[:, hi, icx * IC:(icx + 1) * IC],
                                             ps_y[:], bca[:])

                    if "3" not in phases:
                        continue
                    # projection of this i-chunk's rows (y ready for both heads)
                    for tt in range(icx * (IC // 128), (icx + 1) * (IC // 128)):
                        for ec in range(DM // 512):
                            ps_out = pso.tile([128, 512], F32, tag="pso")
                            for dt_ in range(HPC):
                                nc.tensor.matmul(ps_out[:],
                                                 y_b[:, dt_, tt * 128:(tt + 1) * 128],
                                                 wp_sb[:, dt_, ec * 512:(ec + 1) * 512],
                                                 start=(dt_ == 0), stop=(dt_ == HPC - 1))
                            o_stage = ostp.tile([128, 512], F32, tag="ost")
                            if (tt * 4 + ec) % 2 == 0:
                                nc.vector.tensor_copy(o_stage[:], ps_out[:])
                            else:
                                nc.scalar.copy(o_stage[:], ps_out[:])
                            r0 = b * T + tt * 128
                            nc.sync.dma_start(out[r0:r0 + 128, ec * 512:(ec + 1) * 512],
                                              o_stage[:])

    nc.compile()
    return nc


def _host_tensors():
    """Core-independent constant inputs."""
    inv_freq = 1.0 / (ROPE_THETA ** (np.arange(0, HD, 2, dtype=np.float64) / HD))
    ang = np.arange(T, dtype=np.float64)[:, None] * inv_freq[None, :]   # [T, 64]
    cos_h = np.cos(ang).T.astype(np.float32)                            # [64, T]
    sin_h = np.sin(ang).T.astype(np.float32)
    cosT = np.concatenate([cos_h, cos_h], axis=0)                       # [128, T]
    # tmp[0:64] = q[64:128] * sinT[64:128] needs -sin there; tmp[64:128]
    # = q[0:64] * sinT[0:64] needs +sin (halves hold identical angles)
    sinT = np.concatenate([sin_h, -sin_h], axis=0)

    ones128 = np.ones((128, 1), dtype=np.float32)
    return cosT, sinT, ones128


def _bias_tiles(h0):
    """[128, HPC, 17, 128] additive pre-scale bias, slot s = (it - jt) + 1.

    Slot 0 (it < jt, fully masked) is all NEG; slot 1 (diagonal) has the
    upper triangle NEG; slots 2.. are pure sqrt(HD)*(alibi - M).
    """
    jj = np.arange(128)[:, None]
    ii = np.arange(128)[None, :]
    rel = (jj - ii).astype(np.float64)          # (jj - ii)
    bias = np.empty((128, HPC, 17, 128), dtype=np.float32)
    for e in range(HPC):
        h = h0 + e
        slope = 2.0 ** (-8.0 * (h + 1) / H)
        bias[:, e, 0, :] = NEG
        for d in range(16):                      # d = it - jt >= 0
            v = SQHD * (slope * (rel - 128.0 * d) - M_OFF)
            tile_v = v.astype(np.float32)
            if d == 0:
                tile_v = np.where(jj > ii, NEG, tile_v)
            bias[:, e, d + 1, :] = tile_v
    return bias


_NC_CACHE = {}


def _get_program():
    if "nc" not in _NC_CACHE:
        _NC_CACHE["nc"] = build_program()
    return _NC_CACHE["nc"]


def make_in_maps(x, W_qkv, W_proj):
    x = np.asarray(x, dtype=np.float32)
    W_qkv = np.asarray(W_qkv, dtype=np.float32)
    W_proj = np.asarray(W_proj, dtype=np.float32)

    bf16 = mybir.dt.np(BF16)
    xT = np.ascontiguousarray(x.reshape(ROWS, DM).T).astype(bf16)   # [DM, ROWS]
    Wq, Wk, Wv = W_qkv[:, :DM], W_qkv[:, DM:2 * DM], W_qkv[:, 2 * DM:]
    cosT, sinT, ones128 = _host_tensors()

    in_maps = []
    for c in range(NCORES):
        h0 = HPC * c
        cols = np.r_[h0 * HD:(h0 + 1) * HD, (h0 + 1) * HD:(h0 + 2) * HD]
        in_maps.append({
            "xT": xT,
            "wq": np.ascontiguousarray(Wq[:, cols]).astype(bf16),
            "wk": np.ascontiguousarray(Wk[:, cols]).astype(bf16),
            "wv": np.ascontiguousarray(Wv[:, cols]).astype(bf16),
            "wp": np.ascontiguousarray(W_proj[cols, :]),
            "cosT": cosT,
            "sinT": sinT,
            "biasd": _bias_tiles(h0),
            "ones128": ones128,
        })
    return in_maps


def kernel(x, causal_mask, W_qkv, W_proj):
    del causal_mask  # always lower-triangular; causality is hardcoded
    nc = _get_program()
    in_maps = make_in_maps(x, W_qkv, W_proj)
    res = run_bass_kernel_spmd(nc, in_maps, core_ids=list(range(NCORES)))
    acc = np.zeros((ROWS, DM), dtype=np.float32)
    for c in range(NCORES):
        acc += res.results[c]["out"]
    return acc.reshape(B, T, DM)

